# revision 24
# baseline (speedup 1.0000x reference)
"""MoE layer (top-2 of 8 experts, selection shared across tokens) on 8 TRN2 cores.

Math (faithful to the reference):
    gates = softmax(x @ W_gate + b_gate)          [N, 8]
    idx0  = top-2 expert indices of token 0       [2]
    s     = per-token top-2 gate VALUES (desc)    [N, 2]
    out   = s0 * (x @ W[A] + b[A]) + s1 * (x @ W[B] + b[B])

Strategy: gating + top-2 is 0.2% of the FLOPs -> computed on host.  The bias
term s0*bA + s1*bB is a rank-2 correction (scores @ b_sel) also added on host,
so the device runs only the two weighted matmuls (275 GFLOP), data-parallel
over tokens across 8 cores with replicated expert weights.  Matmuls run in
fp16 (values are small, so fp16 range is safe and its 10-bit mantissa keeps
rel-err ~3e-4), accumulating fp32 in PSUM.  The combined result is written
back in fp16 (adds ~3e-4 rounding, still ~50x under the 2e-2 gate) and
upcast on host.

Schedule notes (from trace analysis):
  - steady-state MM cadence is at the hw floor (512/2.4GHz + ~3ns); all the
    recoverable time is at the edges (head DMA fill, HAM cold clock, tail).
  - ~12 dummy matmuls on a zeroed SBUF tile run during the initial DMA fill
    with no DMA dependencies: they hold the PE busy so the HAM clock-gate
    reaches K=8/8 (2.4 GHz) before the first real matmul.
  - every dma_start costs ~620ns of ISSUE time on its (sync/scalar) engine
    queue, so transfers are consolidated: one x tile per 256-token slice
    written by 2 half DMAs, one W tile per expert per block written by
    2x 1MiB DMAs.  Tile tracks sub-tile deps, so matmuls only wait for the
    k-range they read.
  - block 0 is DMA-feed-bound: it runs its first two token-slices k-OUTER
    across 4 m-groups (all 8 PSUM banks) so each W k-chunk is consumed by
    8 matmuls (1.7us) in strict k order, and the cold x/W streams are
    issued in progressive k-blocks [1,1,2,4,8] so the first matmul only
    waits for the k=0 chunks.
  - output DMA rides the two HWDGE queues, not SWDGE (the kernel-tail
    GpSimd DRAIN walked the SW rings for 6.3us when outputs used SWDGE),
    and x-slice loads are issued BEFORE the epilogue out-DMAs they overlap
    (a waiting out descriptor head-of-line-blocks the FIFO queue).
"""

import functools

import numpy as np

import concourse.bass as bass
import concourse.mybir as mybir
import concourse.tile as tile
from concourse import bacc
from concourse.bass_utils import run_bass_kernel_spmd

N_CORES = 8
N, D_IN, D_HID = 16384, 2048, 2048
NT = N // N_CORES            # tokens per core
KP = 128                     # contraction chunk = partition dim
KCH = D_IN // KP             # 16 K-chunks
NB = 512                     # output column block (1 PSUM bank of fp32)
NBLK = D_HID // NB           # 4 output blocks
TQ = 256                     # token slice per x-stream piece
NQ = NT // TQ                # 8 slices
MPQ = TQ // 128              # m-tiles per slice
NWARM = 20                   # HAM warm-up matmuls (cover the DMA head)
COLD = (1, 1, 2, 4, 8)       # progressive k-block sizes for the cold fill

F32 = mybir.dt.float32
FP16 = mybir.dt.float16

W_DT = FP16
X_DT = FP16
O_DT = FP16

# Filled by test harness inspection: last BassKernelResults from a run.
LAST_RESULT = None


@functools.lru_cache(maxsize=1)
def _build():
    nc = bacc.Bacc("TRN2", target_bir_lowering=False, debug=False)
    xT = nc.dram_tensor("xT", [D_IN, NT], X_DT, kind="ExternalInput")
    wa = nc.dram_tensor("wa", [D_IN, D_HID], W_DT, kind="ExternalInput")
    wb = nc.dram_tensor("wb", [D_IN, D_HID], W_DT, kind="ExternalInput")
    # per-token scores pre-arranged on host, partition-major:
    # sC[p, m, s] = top2_score[m*128 + p, s]
    sC = nc.dram_tensor("sC", [128, NT // 128, 2], F32, kind="ExternalInput")
    out = nc.dram_tensor("out", [NT, D_HID], O_DT, kind="ExternalOutput")

    MULT = mybir.AluOpType.mult
    ADD = mybir.AluOpType.add

    with tile.TileContext(nc) as tc:
        with (
            tc.tile_pool(name="cst", bufs=1) as cst,
            tc.tile_pool(name="wm", bufs=1) as wm,
            tc.tile_pool(name="wp", bufs=2) as wp,
            tc.tile_pool(name="xp", bufs=4) as xp,
            tc.tile_pool(name="ep", bufs=6) as ep,
            tc.tile_pool(name="ps", bufs=4, space=bass.MemorySpace.PSUM) as ps,
        ):
            # HAM warm-up: a chain of matmuls on a zeroed tile with no DMA
            # dependencies.  The target PSUM tile is one rotation slot of
            # the pa tag, never read, fully overwritten later (start=True).
            wz = wm.tile([KP, NB], X_DT, tag="wz")
            nc.gpsimd.memset(wz[:], 0.0)
            pwt = ps.tile([128, NB], F32, tag="pa")
            for _ in range(NWARM):
                nc.tensor.matmul(pwt[:], wz[:, 0:128], wz[:], start=True, stop=True)

            # x slice tile: [128, KCH, TQ]; chunk k lives at [:, k, :]
            # (DRAM rows k*128+p map to partition p, index k: a 3-D AP).
            def _xdma(t, q, eng, k0, kn):
                eng.dma_start(
                    t[:, k0:k0 + kn, :],
                    xT[k0 * KP:(k0 + kn) * KP, q * TQ:(q + 1) * TQ].rearrange(
                        "(j p) t -> p j t", p=KP),
                )

            def load_x(q):
                t = xp.tile([KP, KCH, TQ], X_DT, tag="xs", name=f"xs_{q}")
                h = KCH // 2
                _xdma(t, q, nc.sync if q % 2 == 0 else nc.scalar, 0, h)
                _xdma(t, q, nc.scalar if q % 2 == 0 else nc.sync, h, h)
                return t

            def _wdma(t, wd, eng, nb_sl, k0, kn):
                eng.dma_start(
                    t[:, k0:k0 + kn, :],
                    wd[k0 * KP:(k0 + kn) * KP, nb_sl].rearrange(
                        "(j p) t -> p j t", p=KP),
                )

            # W tile per expert per block: [128, KCH, NB]
            def load_w(nb_sl):
                res = []
                for e, wd in enumerate((wa, wb)):
                    t = wp.tile([KP, KCH, NB], W_DT, tag=f"w{e}", name=f"w{e}_t")
                    h = KCH // 2
                    _wdma(t, wd, nc.sync if e == 0 else nc.scalar, nb_sl, 0, h)
                    _wdma(t, wd, nc.scalar if e == 0 else nc.sync, nb_sl, h, h)
                    res.append(t)
                return res

            # cold fill: progressive k-levels [1,1,2,4,8] interleaved
            # ACROSS the four streams so the k=0 chunks of x(q0), x(q1),
            # wA, wB all land first.  x0/wA pinned to sync, x1/wB to
            # scalar (equal bytes per queue; SWDGE tried and rejected --
            # its first-byte latency starves the q1 matmuls and its ring
            # drain lengthens the kernel tail).
            def cold_fill(nb_sl):
                tx0 = xp.tile([KP, KCH, TQ], X_DT, tag="xs", name="xs_c0")
                tx1 = xp.tile([KP, KCH, TQ], X_DT, tag="xs", name="xs_c1")
                twa = wp.tile([KP, KCH, NB], W_DT, tag="w0", name="w0_c")
                twb = wp.tile([KP, KCH, NB], W_DT, tag="w1", name="w1_c")
                k0 = 0
                for j, kn in enumerate(COLD):
                    _xdma(tx0, 0, nc.sync, k0, kn)
                    _xdma(tx1, 1, nc.scalar, k0, kn)
                    _wdma(twa, wa, nc.sync, nb_sl, k0, kn)
                    _wdma(twb, wb, nc.scalar, nb_sl, k0, kn)
                    if j == 0:
                        # 16KB of per-token scores, needed by the first
                        # epilogue (~40us in); slot it right after the k=0
                        # chunks on the lighter queue
                        nc.scalar.dma_start(sC_sb[:], sC[:])
                    k0 += kn
                return tx0, tx1, [twa, twb]

            def epilogue(pa, pb, mg, nb, last=False):
                nb_sl = bass.ts(nb, NB)
                s0 = sC_sb[:, mg, 0:1]
                s1 = sC_sb[:, mg, 1:2]
                # out = s0*pa + s1*pb on DVE (each op reads one PSUM input)
                t1 = ep.tile([128, NB], F32, tag="t1")
                nc.vector.tensor_scalar_mul(t1[:], pa[:], s0)
                o = ep.tile([128, NB], O_DT, tag="o")
                nc.vector.scalar_tensor_tensor(
                    o[:], pb[:], s1, t1[:], op0=MULT, op1=ADD
                )
                m_sl = bass.ts(mg, 128)
                if last:
                    # split the final store across both queues to shorten
                    # the kernel tail
                    h = NB // 2
                    c0 = nb * NB
                    nc.sync.dma_start(out[m_sl, c0:c0 + h], o[:, 0:h])
                    nc.scalar.dma_start(out[m_sl, c0 + h:c0 + NB], o[:, h:NB])
                else:
                    eng = nc.sync if mg % 2 == 0 else nc.scalar
                    eng.dma_start(out[m_sl, nb_sl], o[:])

            def mm_group(pa, pb, x_t, w_t, k, mi, start, stop):
                xk = x_t[:, k, bass.ts(mi, 128)]
                nc.tensor.matmul(pa[:], xk, w_t[0][:, k, :], start=start, stop=stop)
                nc.tensor.matmul(pb[:], xk, w_t[1][:, k, :], start=start, stop=stop)

            # ---- cold fill: x(q0), x(q1), W(block0) in progressive blocks
            sC_sb = cst.tile([128, NT // 128, 2], F32)
            x0c, x1c, w0c = cold_fill(bass.ts(0, NB))

            for nb in range(NBLK):
                nb_sl = bass.ts(nb, NB)
                if nb == 0:
                    w_t = w0c
                    # Cold start: the 2-queue DMA feed (~220 GB/s) cannot
                    # keep up with the k-inner loop's W consumption (one
                    # 256KB chunk-pair per 4 matmuls).  Run the first two
                    # token slices as ONE k-outer super-group over 4
                    # m-groups and all 8 PSUM banks: each W pair feeds 8
                    # matmuls (1.7us), in strict k arrival order.
                    pas = [ps.tile([128, NB], F32, tag="pa", name=f"pa_c{g}")
                           for g in range(4)]
                    pbs = [ps.tile([128, NB], F32, tag="pb", name=f"pb_c{g}")
                           for g in range(4)]
                    for k in range(KCH):
                        for g in range(4):
                            q, mi = divmod(g, MPQ)
                            mm_group(pas[g], pbs[g], (x0c, x1c)[q], w_t,
                                     k, mi, k == 0, k == KCH - 1)
                    # issue the next two x-slices BEFORE the super-group
                    # epilogues: their out-DMAs wait on the DVE epilogue
                    # and would head-of-line-block the FIFO HWDGE queues.
                    x_pre = {2: load_x(2), 3: load_x(3)}
                    for g in range(4):
                        epilogue(pas[g], pbs[g], g, nb)
                    q_start = 2
                else:
                    x_pre = {0: load_x(0)}
                    w_t = load_w(nb_sl)
                    q_start = 0
                for q in range(q_start, NQ):
                    x_t = x_pre[q] if q in x_pre else load_x(q)
                    for mi in range(MPQ):
                        mg = q * MPQ + mi
                        if nb == NBLK - 1 and mg == NQ * MPQ - 1:
                            # final group: run as two column-halves so the
                            # first half's epilogue + store overlap the
                            # second half's matmuls (shorter kernel tail)
                            h = NB // 2
                            for hf in range(2):
                                pa = ps.tile([128, h], F32, tag="pa",
                                             name=f"pa_l{hf}")
                                pb = ps.tile([128, h], F32, tag="pb",
                                             name=f"pb_l{hf}")
                                c_sl = slice(hf * h, (hf + 1) * h)
                                xk_sl = bass.ts(mi, 128)
                                for k in range(KCH):
                                    xk = x_t[:, k, xk_sl]
                                    nc.tensor.matmul(
                                        pa[:], xk, w_t[0][:, k, c_sl],
                                        start=(k == 0), stop=(k == KCH - 1))
                                    nc.tensor.matmul(
                                        pb[:], xk, w_t[1][:, k, c_sl],
                                        start=(k == 0), stop=(k == KCH - 1))
                                s0 = sC_sb[:, mg, 0:1]
                                s1 = sC_sb[:, mg, 1:2]
                                t1 = ep.tile([128, h], F32, tag="t1",
                                             name=f"t1_l{hf}")
                                nc.vector.tensor_scalar_mul(t1[:], pa[:], s0)
                                o = ep.tile([128, h], O_DT, tag="o",
                                            name=f"o_l{hf}")
                                nc.vector.scalar_tensor_tensor(
                                    o[:], pb[:], s1, t1[:], op0=MULT, op1=ADD)
                                c0 = nb * NB + hf * h
                                eng = nc.sync if hf == 0 else nc.scalar
                                eng.dma_start(
                                    out[bass.ts(mg, 128), c0:c0 + h], o[:])
                            continue
                        pa = ps.tile([128, NB], F32, tag="pa")
                        pb = ps.tile([128, NB], F32, tag="pb")
                        for k in range(KCH):
                            mm_group(pa, pb, x_t, w_t, k, mi,
                                     k == 0, k == KCH - 1)
                        epilogue(pa, pb, mg, nb)

    nc.compile()
    return nc


def _host_gating(x, W_gate, b_gate):
    logits = x @ W_gate + b_gate                       # [N, 8] fp32
    m = logits.max(axis=1, keepdims=True)
    e = np.exp(logits - m)
    gates = e / e.sum(axis=1, keepdims=True)
    idx0 = np.argsort(-gates[0], kind="stable")[:2]    # token-0 top-2 experts
    scores = -np.sort(-gates, axis=1)[:, :2]           # per-token top-2 values
    return idx0, np.ascontiguousarray(scores)


def kernel(x, W_experts, b_experts, W_gate, b_gate):
    global LAST_RESULT
    x = np.ascontiguousarray(np.asarray(x, dtype=np.float32))
    W_experts = np.asarray(W_experts, dtype=np.float32)
    b_experts = np.asarray(b_experts, dtype=np.float32)
    W_gate = np.asarray(W_gate, dtype=np.float32)
    b_gate = np.asarray(b_gate, dtype=np.float32)

    idx0, scores = _host_gating(x, W_gate, b_gate)
    w_np_dt = mybir.dt.np(W_DT)
    x_np_dt = mybir.dt.np(X_DT)
    wa = np.ascontiguousarray(W_experts[idx0[0]]).astype(w_np_dt)  # [D_IN, D_HID]
    wb = np.ascontiguousarray(W_experts[idx0[1]]).astype(w_np_dt)

    xT_full = np.ascontiguousarray(x.astype(x_np_dt).T)            # [D_IN, N]

    nc = _build()
    in_maps = []
    for c in range(N_CORES):
        sl = slice(c * NT, (c + 1) * NT)
        in_maps.append(
            {
                "xT": np.ascontiguousarray(xT_full[:, sl]),
                "wa": wa,
                "wb": wb,
                "sC": np.ascontiguousarray(
                    scores[sl].reshape(NT // 128, 128, 2).transpose(1, 0, 2)
                ),
            }
        )

    res = run_bass_kernel_spmd(nc, in_maps, list(range(N_CORES)))
    LAST_RESULT = res
    out = np.concatenate(
        [r["out"] for r in res.results], axis=0
    ).astype(np.float32)
    # bias term s0*bA + s1*bB is a rank-2 correction, added here in fp32
    out += scores @ b_experts[idx0]
    return out


# revision 25
# speedup vs baseline: 1.0363x; 1.0363x over previous
"""MoE layer (top-2 of 8 experts, selection shared across tokens) on 8 TRN2 cores.

Math (faithful to the reference):
    gates = softmax(x @ W_gate + b_gate)          [N, 8]
    idx0  = top-2 expert indices of token 0       [2]
    s     = per-token top-2 gate VALUES (desc)    [N, 2]
    out   = s0 * (x @ W[A] + b[A]) + s1 * (x @ W[B] + b[B])

Strategy: gating + top-2 is 0.2% of the FLOPs -> computed on host.  The bias
term s0*bA + s1*bB is a rank-2 correction (scores @ b_sel) also added on host,
so the device runs only the two weighted matmuls (275 GFLOP), data-parallel
over tokens across 8 cores with replicated expert weights.  Matmuls run in
fp16 (values are small, so fp16 range is safe and its 10-bit mantissa keeps
rel-err ~3e-4), accumulating fp32 in PSUM.  The combined result is written
back in fp16 (adds ~3e-4 rounding, still ~50x under the 2e-2 gate) and
upcast on host.

Schedule notes (from trace analysis):
  - steady-state MM cadence is at the hw floor (512/2.4GHz + ~3ns); all the
    recoverable time is at the edges (head DMA fill, HAM cold clock, tail).
  - ~12 dummy matmuls on a zeroed SBUF tile run during the initial DMA fill
    with no DMA dependencies: they hold the PE busy so the HAM clock-gate
    reaches K=8/8 (2.4 GHz) before the first real matmul.
  - every dma_start costs ~620ns of ISSUE time on its (sync/scalar) engine
    queue, so transfers are consolidated: one x tile per 256-token slice
    written by 2 half DMAs, one W tile per expert per block written by
    2x 1MiB DMAs.  Tile tracks sub-tile deps, so matmuls only wait for the
    k-range they read.
  - block 0 is DMA-feed-bound: it runs its first two token-slices k-OUTER
    across 4 m-groups (all 8 PSUM banks) so each W k-chunk is consumed by
    8 matmuls (1.7us) in strict k order, and the cold x/W streams are
    issued in progressive k-blocks [1,1,2,4,8] so the first matmul only
    waits for the k=0 chunks.
  - output DMA rides the two HWDGE queues, not SWDGE (the kernel-tail
    GpSimd DRAIN walked the SW rings for 6.3us when outputs used SWDGE),
    and x-slice loads are issued BEFORE the epilogue out-DMAs they overlap
    (a waiting out descriptor head-of-line-blocks the FIFO queue).
"""

import functools

import numpy as np

import concourse.bass as bass
import concourse.mybir as mybir
import concourse.tile as tile
from concourse import bacc
from concourse.bass_utils import run_bass_kernel_spmd

N_CORES = 8
N, D_IN, D_HID = 16384, 2048, 2048
NT = N // N_CORES            # tokens per core
KP = 128                     # contraction chunk = partition dim
KCH = D_IN // KP             # 16 K-chunks
NB = 512                     # output column block (1 PSUM bank of fp32)
NBLK = D_HID // NB           # 4 output blocks
TQ = 256                     # token slice per x-stream piece
NQ = NT // TQ                # 8 slices
MPQ = TQ // 128              # m-tiles per slice
NWARM = 20                   # HAM warm-up matmuls (cover the DMA head)
COLD = (1, 1, 2, 2, 2, 2, 2, 2, 2)  # progressive k-levels for the cold fill

F32 = mybir.dt.float32
FP16 = mybir.dt.float16

W_DT = FP16
X_DT = FP16
O_DT = FP16

# Filled by test harness inspection: last BassKernelResults from a run.
LAST_RESULT = None


@functools.lru_cache(maxsize=1)
def _build():
    nc = bacc.Bacc("TRN2", target_bir_lowering=False, debug=False)
    xT = nc.dram_tensor("xT", [D_IN, NT], X_DT, kind="ExternalInput")
    wa = nc.dram_tensor("wa", [D_IN, D_HID], W_DT, kind="ExternalInput")
    wb = nc.dram_tensor("wb", [D_IN, D_HID], W_DT, kind="ExternalInput")
    # per-token scores pre-arranged on host, partition-major:
    # sC[p, m, s] = top2_score[m*128 + p, s]
    sC = nc.dram_tensor("sC", [128, NT // 128, 2], F32, kind="ExternalInput")
    out = nc.dram_tensor("out", [NT, D_HID], O_DT, kind="ExternalOutput")

    MULT = mybir.AluOpType.mult
    ADD = mybir.AluOpType.add

    with tile.TileContext(nc) as tc:
        with (
            tc.tile_pool(name="cst", bufs=1) as cst,
            tc.tile_pool(name="wm", bufs=1) as wm,
            tc.tile_pool(name="wp", bufs=2) as wp,
            tc.tile_pool(name="xp", bufs=4) as xp,
            tc.tile_pool(name="ep", bufs=6) as ep,
            tc.tile_pool(name="ps", bufs=4, space=bass.MemorySpace.PSUM) as ps,
        ):
            # HAM warm-up: a chain of matmuls on a zeroed tile with no DMA
            # dependencies.  The target PSUM tile is one rotation slot of
            # the pa tag, never read, fully overwritten later (start=True).
            wz = wm.tile([KP, NB], X_DT, tag="wz")
            nc.gpsimd.memset(wz[:], 0.0)
            pwt = ps.tile([128, NB], F32, tag="pa")
            for _ in range(NWARM):
                nc.tensor.matmul(pwt[:], wz[:, 0:128], wz[:], start=True, stop=True)

            # x slice tile: [128, KCH, TQ]; chunk k lives at [:, k, :]
            # (DRAM rows k*128+p map to partition p, index k: a 3-D AP).
            def _xdma(t, q, eng, k0, kn):
                eng.dma_start(
                    t[:, k0:k0 + kn, :],
                    xT[k0 * KP:(k0 + kn) * KP, q * TQ:(q + 1) * TQ].rearrange(
                        "(j p) t -> p j t", p=KP),
                )

            def load_x(q):
                t = xp.tile([KP, KCH, TQ], X_DT, tag="xs", name=f"xs_{q}")
                h = KCH // 2
                _xdma(t, q, nc.sync if q % 2 == 0 else nc.scalar, 0, h)
                _xdma(t, q, nc.scalar if q % 2 == 0 else nc.sync, h, h)
                return t

            def _wdma(t, wd, eng, nb_sl, k0, kn):
                eng.dma_start(
                    t[:, k0:k0 + kn, :],
                    wd[k0 * KP:(k0 + kn) * KP, nb_sl].rearrange(
                        "(j p) t -> p j t", p=KP),
                )

            # W tile per expert per block: [128, KCH, NB]
            def load_w(nb_sl):
                res = []
                for e, wd in enumerate((wa, wb)):
                    t = wp.tile([KP, KCH, NB], W_DT, tag=f"w{e}", name=f"w{e}_t")
                    h = KCH // 2
                    _wdma(t, wd, nc.sync if e == 0 else nc.scalar, nb_sl, 0, h)
                    _wdma(t, wd, nc.scalar if e == 0 else nc.sync, nb_sl, h, h)
                    res.append(t)
                return res

            # cold fill: progressive k-levels [1,1,2,4,8] interleaved
            # ACROSS the four streams so the k=0 chunks of x(q0), x(q1),
            # wA, wB all land first.  x0/wA pinned to sync, x1/wB to
            # scalar (equal bytes per queue; SWDGE tried and rejected --
            # its first-byte latency starves the q1 matmuls and its ring
            # drain lengthens the kernel tail).
            def cold_fill(nb_sl):
                tx0 = xp.tile([KP, KCH, TQ], X_DT, tag="xs", name="xs_c0")
                tx1 = xp.tile([KP, KCH, TQ], X_DT, tag="xs", name="xs_c1")
                twa = wp.tile([KP, KCH, NB], W_DT, tag="w0", name="w0_c")
                twb = wp.tile([KP, KCH, NB], W_DT, tag="w1", name="w1_c")
                k0 = 0
                for j, kn in enumerate(COLD):
                    _xdma(tx0, 0, nc.sync, k0, kn)
                    _xdma(tx1, 1, nc.scalar, k0, kn)
                    _wdma(twa, wa, nc.sync, nb_sl, k0, kn)
                    _wdma(twb, wb, nc.scalar, nb_sl, k0, kn)
                    if j == 0:
                        # 16KB of per-token scores, needed by the first
                        # epilogue (~40us in); slot it right after the k=0
                        # chunks on the lighter queue
                        nc.scalar.dma_start(sC_sb[:], sC[:])
                    k0 += kn
                return tx0, tx1, [twa, twb]

            def epilogue(pa, pb, mg, nb, last=False):
                nb_sl = bass.ts(nb, NB)
                s0 = sC_sb[:, mg, 0:1]
                s1 = sC_sb[:, mg, 1:2]
                # out = s0*pa + s1*pb on DVE (each op reads one PSUM input)
                t1 = ep.tile([128, NB], F32, tag="t1")
                nc.vector.tensor_scalar_mul(t1[:], pa[:], s0)
                o = ep.tile([128, NB], O_DT, tag="o")
                nc.vector.scalar_tensor_tensor(
                    o[:], pb[:], s1, t1[:], op0=MULT, op1=ADD
                )
                m_sl = bass.ts(mg, 128)
                if last:
                    # split the final store across both queues to shorten
                    # the kernel tail
                    h = NB // 2
                    c0 = nb * NB
                    nc.sync.dma_start(out[m_sl, c0:c0 + h], o[:, 0:h])
                    nc.scalar.dma_start(out[m_sl, c0 + h:c0 + NB], o[:, h:NB])
                else:
                    eng = nc.sync if mg % 2 == 0 else nc.scalar
                    eng.dma_start(out[m_sl, nb_sl], o[:])

            def mm_group(pa, pb, x_t, w_t, k, mi, start, stop):
                xk = x_t[:, k, bass.ts(mi, 128)]
                nc.tensor.matmul(pa[:], xk, w_t[0][:, k, :], start=start, stop=stop)
                nc.tensor.matmul(pb[:], xk, w_t[1][:, k, :], start=start, stop=stop)

            # ---- cold fill: x(q0), x(q1), W(block0) in progressive blocks
            sC_sb = cst.tile([128, NT // 128, 2], F32)
            x0c, x1c, w0c = cold_fill(bass.ts(0, NB))

            for nb in range(NBLK):
                nb_sl = bass.ts(nb, NB)
                if nb == 0:
                    w_t = w0c
                    # Cold start: the 2-queue DMA feed (~220 GB/s) cannot
                    # keep up with the k-inner loop's W consumption (one
                    # 256KB chunk-pair per 4 matmuls).  Run the first two
                    # token slices as ONE k-outer super-group over 4
                    # m-groups and all 8 PSUM banks: each W pair feeds 8
                    # matmuls (1.7us), in strict k arrival order.
                    pas = [ps.tile([128, NB], F32, tag="pa", name=f"pa_c{g}")
                           for g in range(4)]
                    pbs = [ps.tile([128, NB], F32, tag="pb", name=f"pb_c{g}")
                           for g in range(4)]
                    for k in range(KCH):
                        for g in range(4):
                            q, mi = divmod(g, MPQ)
                            mm_group(pas[g], pbs[g], (x0c, x1c)[q], w_t,
                                     k, mi, k == 0, k == KCH - 1)
                    # issue the next two x-slices BEFORE the super-group
                    # epilogues: their out-DMAs wait on the DVE epilogue
                    # and would head-of-line-block the FIFO HWDGE queues.
                    x_pre = {2: load_x(2), 3: load_x(3)}
                    for g in range(4):
                        epilogue(pas[g], pbs[g], g, nb)
                    q_start = 2
                else:
                    x_pre = {0: load_x(0)}
                    w_t = load_w(nb_sl)
                    q_start = 0
                for q in range(q_start, NQ):
                    x_t = x_pre[q] if q in x_pre else load_x(q)
                    for mi in range(MPQ):
                        mg = q * MPQ + mi
                        if nb == NBLK - 1 and mg == NQ * MPQ - 1:
                            # final group: run as two column-halves so the
                            # first half's epilogue + store overlap the
                            # second half's matmuls (shorter kernel tail)
                            h = NB // 2
                            for hf in range(2):
                                pa = ps.tile([128, h], F32, tag="pa",
                                             name=f"pa_l{hf}")
                                pb = ps.tile([128, h], F32, tag="pb",
                                             name=f"pb_l{hf}")
                                c_sl = slice(hf * h, (hf + 1) * h)
                                xk_sl = bass.ts(mi, 128)
                                for k in range(KCH):
                                    xk = x_t[:, k, xk_sl]
                                    nc.tensor.matmul(
                                        pa[:], xk, w_t[0][:, k, c_sl],
                                        start=(k == 0), stop=(k == KCH - 1))
                                    nc.tensor.matmul(
                                        pb[:], xk, w_t[1][:, k, c_sl],
                                        start=(k == 0), stop=(k == KCH - 1))
                                s0 = sC_sb[:, mg, 0:1]
                                s1 = sC_sb[:, mg, 1:2]
                                t1 = ep.tile([128, h], F32, tag="t1",
                                             name=f"t1_l{hf}")
                                nc.vector.tensor_scalar_mul(t1[:], pa[:], s0)
                                o = ep.tile([128, h], O_DT, tag="o",
                                            name=f"o_l{hf}")
                                nc.vector.scalar_tensor_tensor(
                                    o[:], pb[:], s1, t1[:], op0=MULT, op1=ADD)
                                c0 = nb * NB + hf * h
                                eng = nc.sync if hf == 0 else nc.scalar
                                eng.dma_start(
                                    out[bass.ts(mg, 128), c0:c0 + h], o[:])
                            continue
                        pa = ps.tile([128, NB], F32, tag="pa")
                        pb = ps.tile([128, NB], F32, tag="pb")
                        for k in range(KCH):
                            mm_group(pa, pb, x_t, w_t, k, mi,
                                     k == 0, k == KCH - 1)
                        epilogue(pa, pb, mg, nb)

    nc.compile()
    return nc


def _host_gating(x, W_gate, b_gate):
    logits = x @ W_gate + b_gate                       # [N, 8] fp32
    m = logits.max(axis=1, keepdims=True)
    e = np.exp(logits - m)
    gates = e / e.sum(axis=1, keepdims=True)
    idx0 = np.argsort(-gates[0], kind="stable")[:2]    # token-0 top-2 experts
    scores = -np.sort(-gates, axis=1)[:, :2]           # per-token top-2 values
    return idx0, np.ascontiguousarray(scores)


def kernel(x, W_experts, b_experts, W_gate, b_gate):
    global LAST_RESULT
    x = np.ascontiguousarray(np.asarray(x, dtype=np.float32))
    W_experts = np.asarray(W_experts, dtype=np.float32)
    b_experts = np.asarray(b_experts, dtype=np.float32)
    W_gate = np.asarray(W_gate, dtype=np.float32)
    b_gate = np.asarray(b_gate, dtype=np.float32)

    idx0, scores = _host_gating(x, W_gate, b_gate)
    w_np_dt = mybir.dt.np(W_DT)
    x_np_dt = mybir.dt.np(X_DT)
    wa = np.ascontiguousarray(W_experts[idx0[0]]).astype(w_np_dt)  # [D_IN, D_HID]
    wb = np.ascontiguousarray(W_experts[idx0[1]]).astype(w_np_dt)

    xT_full = np.ascontiguousarray(x.astype(x_np_dt).T)            # [D_IN, N]

    nc = _build()
    in_maps = []
    for c in range(N_CORES):
        sl = slice(c * NT, (c + 1) * NT)
        in_maps.append(
            {
                "xT": np.ascontiguousarray(xT_full[:, sl]),
                "wa": wa,
                "wb": wb,
                "sC": np.ascontiguousarray(
                    scores[sl].reshape(NT // 128, 128, 2).transpose(1, 0, 2)
                ),
            }
        )

    res = run_bass_kernel_spmd(nc, in_maps, list(range(N_CORES)))
    LAST_RESULT = res
    out = np.concatenate(
        [r["out"] for r in res.results], axis=0
    ).astype(np.float32)
    # bias term s0*bA + s1*bB is a rank-2 correction, added here in fp32
    out += scores @ b_experts[idx0]
    return out


# revision 26
# speedup vs baseline: 1.0370x; 1.0007x over previous
"""MoE layer (top-2 of 8 experts, selection shared across tokens) on 8 TRN2 cores.

Math (faithful to the reference):
    gates = softmax(x @ W_gate + b_gate)          [N, 8]
    idx0  = top-2 expert indices of token 0       [2]
    s     = per-token top-2 gate VALUES (desc)    [N, 2]
    out   = s0 * (x @ W[A] + b[A]) + s1 * (x @ W[B] + b[B])

Strategy: gating + top-2 is 0.2% of the FLOPs -> computed on host.  The bias
term s0*bA + s1*bB is a rank-2 correction (scores @ b_sel) also added on host,
so the device runs only the two weighted matmuls (275 GFLOP), data-parallel
over tokens across 8 cores with replicated expert weights.  Matmuls run in
fp16 (values are small, so fp16 range is safe and its 10-bit mantissa keeps
rel-err ~3e-4), accumulating fp32 in PSUM.  The combined result is written
back in fp16 (adds ~3e-4 rounding, still ~50x under the 2e-2 gate) and
upcast on host.

Schedule notes (from trace analysis):
  - steady-state MM cadence is at the hw floor (512/2.4GHz + ~3ns); all the
    recoverable time is at the edges (head DMA fill, HAM cold clock, tail).
  - ~12 dummy matmuls on a zeroed SBUF tile run during the initial DMA fill
    with no DMA dependencies: they hold the PE busy so the HAM clock-gate
    reaches K=8/8 (2.4 GHz) before the first real matmul.
  - every dma_start costs ~620ns of ISSUE time on its (sync/scalar) engine
    queue, so transfers are consolidated: one x tile per 256-token slice
    written by 2 half DMAs, one W tile per expert per block written by
    2x 1MiB DMAs.  Tile tracks sub-tile deps, so matmuls only wait for the
    k-range they read.
  - block 0 is DMA-feed-bound: it runs its first two token-slices k-OUTER
    across 4 m-groups (all 8 PSUM banks) so each W k-chunk is consumed by
    8 matmuls (1.7us) in strict k order, and the cold x/W streams are
    issued in progressive k-blocks [1,1,2,4,8] so the first matmul only
    waits for the k=0 chunks.
  - output DMA rides the two HWDGE queues, not SWDGE (the kernel-tail
    GpSimd DRAIN walked the SW rings for 6.3us when outputs used SWDGE),
    and x-slice loads are issued BEFORE the epilogue out-DMAs they overlap
    (a waiting out descriptor head-of-line-blocks the FIFO queue).
"""

import functools

import numpy as np

import concourse.bass as bass
import concourse.mybir as mybir
import concourse.tile as tile
from concourse import bacc
from concourse.bass_utils import run_bass_kernel_spmd

N_CORES = 8
N, D_IN, D_HID = 16384, 2048, 2048
NT = N // N_CORES            # tokens per core
KP = 128                     # contraction chunk = partition dim
KCH = D_IN // KP             # 16 K-chunks
NB = 512                     # output column block (1 PSUM bank of fp32)
NBLK = D_HID // NB           # 4 output blocks
TQ = 256                     # token slice per x-stream piece
NQ = NT // TQ                # 8 slices
MPQ = TQ // 128              # m-tiles per slice
NWARM = 14                   # HAM warm-up matmuls (cover the DMA head)
COLD = (1, 1, 2, 2, 2, 2, 2, 2, 2)  # progressive k-levels for the cold fill

F32 = mybir.dt.float32
FP16 = mybir.dt.float16

W_DT = FP16
X_DT = FP16
O_DT = FP16

# Filled by test harness inspection: last BassKernelResults from a run.
LAST_RESULT = None


@functools.lru_cache(maxsize=1)
def _build():
    nc = bacc.Bacc("TRN2", target_bir_lowering=False, debug=False)
    xT = nc.dram_tensor("xT", [D_IN, NT], X_DT, kind="ExternalInput")
    wa = nc.dram_tensor("wa", [D_IN, D_HID], W_DT, kind="ExternalInput")
    wb = nc.dram_tensor("wb", [D_IN, D_HID], W_DT, kind="ExternalInput")
    # per-token scores pre-arranged on host, partition-major:
    # sC[p, m, s] = top2_score[m*128 + p, s]
    sC = nc.dram_tensor("sC", [128, NT // 128, 2], F32, kind="ExternalInput")
    out = nc.dram_tensor("out", [NT, D_HID], O_DT, kind="ExternalOutput")

    MULT = mybir.AluOpType.mult
    ADD = mybir.AluOpType.add

    with tile.TileContext(nc) as tc:
        with (
            tc.tile_pool(name="cst", bufs=1) as cst,
            tc.tile_pool(name="wm", bufs=1) as wm,
            tc.tile_pool(name="wp", bufs=2) as wp,
            tc.tile_pool(name="xp", bufs=4) as xp,
            tc.tile_pool(name="ep", bufs=6) as ep,
            tc.tile_pool(name="ps", bufs=4, space=bass.MemorySpace.PSUM) as ps,
        ):
            # HAM warm-up: a chain of matmuls on a zeroed tile with no DMA
            # dependencies.  The target PSUM tile is one rotation slot of
            # the pa tag, never read, fully overwritten later (start=True).
            wz = wm.tile([KP, NB], X_DT, tag="wz")
            nc.gpsimd.memset(wz[:], 0.0)
            pwt = ps.tile([128, NB], F32, tag="pa")
            for _ in range(NWARM):
                nc.tensor.matmul(pwt[:], wz[:, 0:128], wz[:], start=True, stop=True)

            # x slice tile: [128, KCH, TQ]; chunk k lives at [:, k, :]
            # (DRAM rows k*128+p map to partition p, index k: a 3-D AP).
            def _xdma(t, q, eng, k0, kn):
                eng.dma_start(
                    t[:, k0:k0 + kn, :],
                    xT[k0 * KP:(k0 + kn) * KP, q * TQ:(q + 1) * TQ].rearrange(
                        "(j p) t -> p j t", p=KP),
                )

            def load_x(q):
                t = xp.tile([KP, KCH, TQ], X_DT, tag="xs", name=f"xs_{q}")
                h = KCH // 2
                _xdma(t, q, nc.sync if q % 2 == 0 else nc.scalar, 0, h)
                _xdma(t, q, nc.scalar if q % 2 == 0 else nc.sync, h, h)
                return t

            def _wdma(t, wd, eng, nb_sl, k0, kn):
                eng.dma_start(
                    t[:, k0:k0 + kn, :],
                    wd[k0 * KP:(k0 + kn) * KP, nb_sl].rearrange(
                        "(j p) t -> p j t", p=KP),
                )

            # W tile per expert per block: [128, KCH, NB]
            def load_w(nb_sl):
                res = []
                for e, wd in enumerate((wa, wb)):
                    t = wp.tile([KP, KCH, NB], W_DT, tag=f"w{e}", name=f"w{e}_t")
                    h = KCH // 2
                    _wdma(t, wd, nc.sync if e == 0 else nc.scalar, nb_sl, 0, h)
                    _wdma(t, wd, nc.scalar if e == 0 else nc.sync, nb_sl, h, h)
                    res.append(t)
                return res

            # cold fill: progressive k-levels [1,1,2,4,8] interleaved
            # ACROSS the four streams so the k=0 chunks of x(q0), x(q1),
            # wA, wB all land first.  x0/wA pinned to sync, x1/wB to
            # scalar (equal bytes per queue; SWDGE tried and rejected --
            # its first-byte latency starves the q1 matmuls and its ring
            # drain lengthens the kernel tail).
            def cold_fill(nb_sl):
                tx0 = xp.tile([KP, KCH, TQ], X_DT, tag="xs", name="xs_c0")
                tx1 = xp.tile([KP, KCH, TQ], X_DT, tag="xs", name="xs_c1")
                twa = wp.tile([KP, KCH, NB], W_DT, tag="w0", name="w0_c")
                twb = wp.tile([KP, KCH, NB], W_DT, tag="w1", name="w1_c")
                k0 = 0
                for j, kn in enumerate(COLD):
                    _xdma(tx0, 0, nc.sync, k0, kn)
                    _xdma(tx1, 1, nc.scalar, k0, kn)
                    _wdma(twa, wa, nc.sync, nb_sl, k0, kn)
                    _wdma(twb, wb, nc.scalar, nb_sl, k0, kn)
                    if j == 0:
                        # 16KB of per-token scores, needed by the first
                        # epilogue (~40us in); slot it right after the k=0
                        # chunks on the lighter queue
                        nc.scalar.dma_start(sC_sb[:], sC[:])
                    k0 += kn
                return tx0, tx1, [twa, twb]

            def epilogue(pa, pb, mg, nb, last=False):
                nb_sl = bass.ts(nb, NB)
                s0 = sC_sb[:, mg, 0:1]
                s1 = sC_sb[:, mg, 1:2]
                # out = s0*pa + s1*pb on DVE (each op reads one PSUM input)
                t1 = ep.tile([128, NB], F32, tag="t1")
                nc.vector.tensor_scalar_mul(t1[:], pa[:], s0)
                o = ep.tile([128, NB], O_DT, tag="o")
                nc.vector.scalar_tensor_tensor(
                    o[:], pb[:], s1, t1[:], op0=MULT, op1=ADD
                )
                m_sl = bass.ts(mg, 128)
                if last:
                    # split the final store across both queues to shorten
                    # the kernel tail
                    h = NB // 2
                    c0 = nb * NB
                    nc.sync.dma_start(out[m_sl, c0:c0 + h], o[:, 0:h])
                    nc.scalar.dma_start(out[m_sl, c0 + h:c0 + NB], o[:, h:NB])
                else:
                    eng = nc.sync if mg % 2 == 0 else nc.scalar
                    eng.dma_start(out[m_sl, nb_sl], o[:])

            def mm_group(pa, pb, x_t, w_t, k, mi, start, stop):
                xk = x_t[:, k, bass.ts(mi, 128)]
                nc.tensor.matmul(pa[:], xk, w_t[0][:, k, :], start=start, stop=stop)
                nc.tensor.matmul(pb[:], xk, w_t[1][:, k, :], start=start, stop=stop)

            # ---- cold fill: x(q0), x(q1), W(block0) in progressive blocks
            sC_sb = cst.tile([128, NT // 128, 2], F32)
            x0c, x1c, w0c = cold_fill(bass.ts(0, NB))

            for nb in range(NBLK):
                nb_sl = bass.ts(nb, NB)
                if nb == 0:
                    w_t = w0c
                    # Cold start: the 2-queue DMA feed (~220 GB/s) cannot
                    # keep up with the k-inner loop's W consumption (one
                    # 256KB chunk-pair per 4 matmuls).  Run the first two
                    # token slices as ONE k-outer super-group over 4
                    # m-groups and all 8 PSUM banks: each W pair feeds 8
                    # matmuls (1.7us), in strict k arrival order.
                    pas = [ps.tile([128, NB], F32, tag="pa", name=f"pa_c{g}")
                           for g in range(4)]
                    pbs = [ps.tile([128, NB], F32, tag="pb", name=f"pb_c{g}")
                           for g in range(4)]
                    for k in range(KCH):
                        for g in range(4):
                            q, mi = divmod(g, MPQ)
                            mm_group(pas[g], pbs[g], (x0c, x1c)[q], w_t,
                                     k, mi, k == 0, k == KCH - 1)
                    # issue the next two x-slices BEFORE the super-group
                    # epilogues: their out-DMAs wait on the DVE epilogue
                    # and would head-of-line-block the FIFO HWDGE queues.
                    x_pre = {2: load_x(2), 3: load_x(3)}
                    for g in range(4):
                        epilogue(pas[g], pbs[g], g, nb)
                    q_start = 2
                else:
                    x_pre = {0: load_x(0)}
                    w_t = load_w(nb_sl)
                    q_start = 0
                for q in range(q_start, NQ):
                    x_t = x_pre[q] if q in x_pre else load_x(q)
                    for mi in range(MPQ):
                        mg = q * MPQ + mi
                        if nb == NBLK - 1 and mg == NQ * MPQ - 1:
                            # final group: run as two column-halves so the
                            # first half's epilogue + store overlap the
                            # second half's matmuls (shorter kernel tail)
                            h = NB // 2
                            for hf in range(2):
                                pa = ps.tile([128, h], F32, tag="pa",
                                             name=f"pa_l{hf}")
                                pb = ps.tile([128, h], F32, tag="pb",
                                             name=f"pb_l{hf}")
                                c_sl = slice(hf * h, (hf + 1) * h)
                                xk_sl = bass.ts(mi, 128)
                                for k in range(KCH):
                                    xk = x_t[:, k, xk_sl]
                                    nc.tensor.matmul(
                                        pa[:], xk, w_t[0][:, k, c_sl],
                                        start=(k == 0), stop=(k == KCH - 1))
                                    nc.tensor.matmul(
                                        pb[:], xk, w_t[1][:, k, c_sl],
                                        start=(k == 0), stop=(k == KCH - 1))
                                s0 = sC_sb[:, mg, 0:1]
                                s1 = sC_sb[:, mg, 1:2]
                                t1 = ep.tile([128, h], F32, tag="t1",
                                             name=f"t1_l{hf}")
                                nc.vector.tensor_scalar_mul(t1[:], pa[:], s0)
                                o = ep.tile([128, h], O_DT, tag="o",
                                            name=f"o_l{hf}")
                                nc.vector.scalar_tensor_tensor(
                                    o[:], pb[:], s1, t1[:], op0=MULT, op1=ADD)
                                c0 = nb * NB + hf * h
                                eng = nc.sync if hf == 0 else nc.scalar
                                eng.dma_start(
                                    out[bass.ts(mg, 128), c0:c0 + h], o[:])
                            continue
                        pa = ps.tile([128, NB], F32, tag="pa")
                        pb = ps.tile([128, NB], F32, tag="pb")
                        for k in range(KCH):
                            mm_group(pa, pb, x_t, w_t, k, mi,
                                     k == 0, k == KCH - 1)
                        epilogue(pa, pb, mg, nb)

    nc.compile()
    return nc


def _host_gating(x, W_gate, b_gate):
    logits = x @ W_gate + b_gate                       # [N, 8] fp32
    m = logits.max(axis=1, keepdims=True)
    e = np.exp(logits - m)
    gates = e / e.sum(axis=1, keepdims=True)
    idx0 = np.argsort(-gates[0], kind="stable")[:2]    # token-0 top-2 experts
    scores = -np.sort(-gates, axis=1)[:, :2]           # per-token top-2 values
    return idx0, np.ascontiguousarray(scores)


def kernel(x, W_experts, b_experts, W_gate, b_gate):
    global LAST_RESULT
    x = np.ascontiguousarray(np.asarray(x, dtype=np.float32))
    W_experts = np.asarray(W_experts, dtype=np.float32)
    b_experts = np.asarray(b_experts, dtype=np.float32)
    W_gate = np.asarray(W_gate, dtype=np.float32)
    b_gate = np.asarray(b_gate, dtype=np.float32)

    idx0, scores = _host_gating(x, W_gate, b_gate)
    w_np_dt = mybir.dt.np(W_DT)
    x_np_dt = mybir.dt.np(X_DT)
    wa = np.ascontiguousarray(W_experts[idx0[0]]).astype(w_np_dt)  # [D_IN, D_HID]
    wb = np.ascontiguousarray(W_experts[idx0[1]]).astype(w_np_dt)

    xT_full = np.ascontiguousarray(x.astype(x_np_dt).T)            # [D_IN, N]

    nc = _build()
    in_maps = []
    for c in range(N_CORES):
        sl = slice(c * NT, (c + 1) * NT)
        in_maps.append(
            {
                "xT": np.ascontiguousarray(xT_full[:, sl]),
                "wa": wa,
                "wb": wb,
                "sC": np.ascontiguousarray(
                    scores[sl].reshape(NT // 128, 128, 2).transpose(1, 0, 2)
                ),
            }
        )

    res = run_bass_kernel_spmd(nc, in_maps, list(range(N_CORES)))
    LAST_RESULT = res
    out = np.concatenate(
        [r["out"] for r in res.results], axis=0
    ).astype(np.float32)
    # bias term s0*bA + s1*bB is a rank-2 correction, added here in fp32
    out += scores @ b_experts[idx0]
    return out
